# revision 4
# baseline (speedup 1.0000x reference)
"""APPNP propagation (10 hops) on Trainium2, 8 NeuronCores.

Strategy (dst-sharded message passing, deep-pipelined):
- Nodes are sharded over 8 cores by id (6250 dst nodes each). Each core owns
  the incoming edges of its nodes and computes their feature updates.
- Each shard's nodes are split by in-shard id into two halves (3125 nodes),
  packed into tiles 0-24 (half 0) and 25-49 (half 1). The replicated bf16
  "scaled feature" table t[n] = d[n] * feat[n] is split into two pieces
  (one per half, 8*3200 = 25600 rows each, so rows fit int16), AllGathered
  separately: piece 0 fires mid-hop (hidden behind the tail chunks), piece 1
  is the half-size hop tail and overlaps the next hop's half-0 gathers.
- Per hop each core gathers t[src] rows for its edges with 4 dma_gather
  calls per chunk (half-0 edges split across queues 0/1, half-1 across 2/3)
  so all 4 SWDGE queues stay busy, multiplies by per-edge one-hot fp8 weight
  blocks on the PE (segment-sum into PSUM), applies the APPNP update with two
  fused scalar_tensor_tensor ops on the DVE, and stages d*feat (bf16) via the
  otherwise-idle Scalar engine, DMAing each chunk's rows to the AllGather
  input buffer as soon as they are ready.
- Per-core dst tiles are packed so every tile has exactly BA blocks of "A"
  edges (src in half 0) and BB blocks of "B" edges; gather indices are int16
  rows into the corresponding table piece.
"""

import contextlib
import sys
import types

sys.path.insert(0, "/opt/trn_rl_repo")

import numpy as np
import ml_dtypes


# ---------------------------------------------------------------------------
# Environment shims (walrus in this container allows only 1 sync wait per CTRL
# instruction; the image's antenv stub lacks the NTFF profile hook).
# ---------------------------------------------------------------------------
def _install_shims():
    import concourse.mybir as mybir
    import concourse.tile as tile_mod
    from concourse.vector_clock import ScopedClock

    if getattr(tile_mod.TileContext, "_appnp_patched", False):
        return

    def _drain_and_barrier(self, tick_clock, wait_clock):
        nc = self.nc
        probe = nc.sync.nop(nofuse=True)
        wait_clock.add_sem_waits(
            probe.ins, ScopedClock({None: tick_clock.global_clock})
        )
        waits = list(probe.ins.sync_info.on_wait) if probe.ins.sync_info else []
        if probe.ins.sync_info:
            probe.ins.sync_info.on_wait = waits[:1]
        for i in range(1, len(waits)):
            extra = nc.sync.nop(nofuse=True)
            if extra.ins.sync_info is None:
                extra.ins.sync_info = mybir.SyncInfo(
                    on_wait=waits[i : i + 1], on_update=[]
                )
            else:
                extra.ins.sync_info.on_wait = waits[i : i + 1]
        nc.sync.drain()
        nc.all_engine_barrier()
        assert self.sems is not None
        popped = nc._tile_sem_poison_stack.pop()
        assert popped is self._sem_poison
        nc.clear_and_free_semaphores(list(self.sems.allocated().values()))
        nc.all_engine_barrier()

    tile_mod.TileContext._drain_and_barrier = _drain_and_barrier
    tile_mod.TileContext._appnp_patched = True

    import antenv

    if "antenv.axon_hooks" not in sys.modules:
        hooks_mod = types.ModuleType("antenv.axon_hooks")
        _HOOK = [None]
        hooks_mod.set_axon_ntff_profile_hook = lambda h: _HOOK.__setitem__(0, h)
        hooks_mod.get_axon_ntff_profile_hook = lambda: _HOOK[0]
        sys.modules["antenv.axon_hooks"] = hooks_mod
        antenv.axon_hooks = hooks_mod
        try:
            from trn_agent_boot.trn_boot import _ntff_profile_via_ctypes

            hooks_mod.set_axon_ntff_profile_hook(
                _ntff_profile_via_ctypes("/opt/axon/libaxon_pjrt.so")
            )
        except Exception:
            pass

    import concourse.bass_utils as bass_utils

    bass_utils.upload_artifacts = lambda tmpdir: f"file://{tmpdir}"


# ---------------------------------------------------------------------------
# Constants
# ---------------------------------------------------------------------------
NCORES = 8
HOPS = 10
ALPHA = 0.1
D = 128
TILES = 50  # dst tiles per core
HALF_TILES = TILES // 2  # tiles per half
TILES_PER_CHUNK = 5
NCHUNK = TILES // TILES_PER_CHUNK
PIECE_ROWS = NCORES * HALF_TILES * 128  # 25600 rows per table piece

# set by bench harness: {"trace": True} -> records exec_time_ns
PROFILE = {}


# ---------------------------------------------------------------------------
# Host-side graph preprocessing (pure index manipulation)
# ---------------------------------------------------------------------------
def _pack_bins(degA, degB, capA, capB, n_bins, cap_nodes=128):
    """Assign nodes to bins, balancing A and B edge counts. Returns
    (tile_of, part_of) or None if infeasible with the given caps."""
    n = len(degA)
    order = np.argsort(-(degA + degB), kind="stable")
    binsA = np.zeros(n_bins, np.int64)
    binsB = np.zeros(n_bins, np.int64)
    binsN = np.zeros(n_bins, np.int64)
    tile_of = np.zeros(n, np.int32)
    part_of = np.zeros(n, np.int32)
    tA = max(1.0, degA.sum() / n_bins)
    tB = max(1.0, degB.sum() / n_bins)
    for node in order:
        a, b = degA[node], degB[node]
        feas = (binsN < cap_nodes) & (binsA + a <= capA) & (binsB + b <= capB)
        if not feas.any():
            return None
        score = np.maximum((binsA + a) / tA, (binsB + b) / tB)
        score[~feas] = np.inf
        t = int(np.argmin(score))
        tile_of[node] = t
        part_of[node] = binsN[t]
        binsA[t] += a
        binsB[t] += b
        binsN[t] += 1
    return tile_of, part_of


def _preprocess(src, dst, n_nodes):
    shard = n_nodes // NCORES  # 6250
    halfsz = shard // 2  # 3125

    node_ids = np.arange(n_nodes)
    dev_of = (node_ids // shard).astype(np.int32)
    inshard = (node_ids % shard).astype(np.int64)
    half_of = (inshard >= halfsz).astype(np.int32)

    e_dev = dev_of[dst]
    e_isB = half_of[src].astype(bool)  # which table piece the src row lives in

    tile_of = np.zeros(n_nodes, np.int32)  # 0..49 (half 1 -> +25)
    part_of = np.zeros(n_nodes, np.int32)
    packs = []  # (dev, half) -> (degA, degB) over that half's 3125 nodes
    for dev in range(NCORES):
        for h in (0, 1):
            m = (e_dev == dev) & (half_of[dst] == h)
            dl = inshard[dst[m]] - h * halfsz  # 0..halfsz-1
            isB = e_isB[m]
            degA = np.bincount(dl[~isB], minlength=halfsz)
            degB = np.bincount(dl[isB], minlength=halfsz)
            packs.append((degA, degB))

    maxA = max(int(np.ceil(p[0].sum() / HALF_TILES)) for p in packs)
    maxB = max(int(np.ceil(p[1].sum() / HALF_TILES)) for p in packs)
    BA = max(1, (maxA + 127) // 128)
    BB = max(1, (maxB + 127) // 128)
    while True:
        ok = True
        for dev in range(NCORES):
            for h in (0, 1):
                degA, degB = packs[dev * 2 + h]
                r = _pack_bins(degA, degB, BA * 128, BB * 128, HALF_TILES)
                if r is None:
                    ok = False
                    break
                base = dev * shard + h * halfsz
                tile_of[base : base + halfsz] = r[0] + h * HALF_TILES
                part_of[base : base + halfsz] = r[1]
            if not ok:
                break
        if ok:
            break
        # couldn't fit: grow the tighter side
        if BA <= BB:
            BA += 1
        else:
            BB += 1

    # table-piece row (partition-major within a shard's half):
    # row = dev*3200 + part*HALF_TILES + (tile mod HALF_TILES)
    row_of = (
        dev_of.astype(np.int64) * (HALF_TILES * 128)
        + part_of.astype(np.int64) * HALF_TILES
        + (tile_of % HALF_TILES)
    )

    BPC = TILES_PER_CHUNK * (BA + BB)
    nblk = NCHUNK * BPC
    tot_slots = nblk * 128
    per_core = []
    e_srow = row_of[src]
    e_tile = tile_of[dst]
    e_part = part_of[dst]
    for dev in range(NCORES):
        m = e_dev == dev
        tiles_ = e_tile[m]
        isB_ = e_isB[m]
        parts_ = e_part[m]
        srows_ = e_srow[m]
        idx_flat = np.zeros(tot_slots, np.int16)
        w_inblock = []
        w_block = []
        w_dstp = []
        for c in range(NCHUNK):
            for half in (0, 1):  # A (src half 0) then B (src half 1) blocks
                nb = BA if half == 0 else BB
                for k in range(TILES_PER_CHUNK):
                    t = c * TILES_PER_CHUNK + k
                    sel = (tiles_ == t) & (isB_ == bool(half))
                    sr = srows_[sel]
                    pp = parts_[sel]
                    cap = nb * 128
                    assert len(sr) <= cap, (dev, t, half, len(sr), cap)
                    if half == 0:
                        b0 = c * BPC + k * BA
                    else:
                        b0 = c * BPC + TILES_PER_CHUNK * BA + k * BB
                    off = b0 * 128
                    idx_flat[off : off + len(sr)] = sr.astype(np.int16)
                    j = np.arange(len(sr))
                    w_inblock.append((j % 128).astype(np.int64))
                    w_block.append(b0 + j // 128)
                    w_dstp.append(pp.astype(np.int64))
        # wrap indices: slot s -> (partition s%16 [replicated x8], col s//16)
        idx_wrap = np.zeros((128, tot_slots // 16), np.int16)
        for p in range(128):
            idx_wrap[p, :] = idx_flat[p % 16 :: 16]
        w = np.zeros((128, nblk * 128), dtype=ml_dtypes.float8_e4m3)
        w_inblock = np.concatenate(w_inblock)
        w_block = np.concatenate(w_block)
        w_dstp = np.concatenate(w_dstp)
        w[w_inblock, w_block * 128 + w_dstp] = 1.0
        per_core.append({"idx": idx_wrap, "w": w})

    meta = dict(
        BA=BA,
        BB=BB,
        BPC=BPC,
        nblk=nblk,
        tot_slots=tot_slots,
        shard=shard,
        tile_of=tile_of,
        part_of=part_of,
        dev_of=dev_of,
    )
    return per_core, meta


# ---------------------------------------------------------------------------
# Bass kernel build
# ---------------------------------------------------------------------------
def _build(meta, n_hops, layer_reg_len):
    import os
    skip_gather = bool(int(os.environ.get("APPNP_SKIP_GATHER", "0")))
    skip_mm = bool(int(os.environ.get("APPNP_SKIP_MM", "0")))
    import concourse.bacc as bacc
    import concourse.mybir as mybir
    import concourse.tile as tile

    f32, f16, fp8, i16 = (
        mybir.dt.float32,
        mybir.dt.bfloat16,
        mybir.dt.float8e4,
        mybir.dt.int16,
    )
    BA, BB, BPC = meta["BA"], meta["BB"], meta["BPC"]
    nblk, tot_slots = meta["nblk"], meta["tot_slots"]
    TPC = TILES_PER_CHUNK
    mul, add = mybir.AluOpType.mult, mybir.AluOpType.add

    nc = bacc.Bacc(
        "TRN2",
        target_bir_lowering=False,
        debug=False,
        num_devices=NCORES,
        num_swdge_queues=4,
    )
    idx_in = nc.declare_dram_parameter("idx", [128, tot_slots // 16], i16, isOutput=False)
    w_in = nc.declare_dram_parameter("w", [128, nblk * 128], fp8, isOutput=False)
    h_in = nc.declare_dram_parameter("h", [128, TILES * D], f32, isOutput=False)
    dcols_in = nc.declare_dram_parameter("dcols", [128, TILES], f32, isOutput=False)
    lr_in = nc.declare_dram_parameter("lr", [1, layer_reg_len], f32, isOutput=False)
    out_feat = nc.declare_dram_parameter("out_feat", [128, TILES * D], f32, isOutput=True)

    # A-gather split across queues 0/1, B across 2/3 (block-aligned halves)
    nA = TPC * BA * 128
    nB = TPC * BB * 128
    a_splits = [((TPC * BA + 1) // 2) * 128]
    b_splits = [((TPC * BB + 1) // 2) * 128]
    gather_plan = []  # (slot_off, n_idxs, piece, queue)
    gather_plan.append((0, a_splits[0], 0, 0))
    gather_plan.append((a_splits[0], nA - a_splits[0], 0, 1))
    gather_plan.append((nA, b_splits[0], 1, 2))
    gather_plan.append((nA + b_splits[0], nB - b_splits[0], 1, 3))

    with tile.TileContext(nc) as tc:
        with (
            tc.tile_pool(name="const", bufs=1) as const,
            tc.tile_pool(name="ebufs", bufs=3) as ebufs,
            tc.tile_pool(name="stg", bufs=2) as stg,
            tc.tile_pool(name="utmps", bufs=4) as utmps,
            tc.tile_pool(name="ps", bufs=6, space="PSUM") as pspool,
            tc.tile_pool(name="dram", bufs=1, space="DRAM") as dram,
        ):
            idx_sb = const.tile([128, tot_slots // 16], i16)
            nc.sync.dma_start(idx_sb[:], idx_in[:])
            w_sb = const.tile([128, nblk * 128], fp8)
            nc.sync.dma_start(w_sb[:], w_in[:])
            feat = const.tile([128, TILES * D], f32)
            nc.sync.dma_start(feat[:], h_in[:])
            dcols = const.tile([128, TILES], f32)
            nc.sync.dma_start(dcols[:], dcols_in[:])
            lr_sb = const.tile([1, layer_reg_len], f32)
            nc.sync.dma_start(lr_sb[:], lr_in[:])

            # broadcast layer_reg across partitions via K=1 matmul with ones
            ones = const.tile([1, 128], f32)
            nc.vector.memset(ones[:], 1.0)
            ps_r = pspool.tile([128, layer_reg_len], f32, tag="psr", bufs=1)
            nc.tensor.matmul(ps_r[:], ones[:], lr_sb[:], start=True, stop=True)
            rA = const.tile([128, layer_reg_len], f32)
            nc.vector.tensor_scalar_mul(rA[:], ps_r[:], 1.0 - ALPHA)
            rH = const.tile([128, layer_reg_len], f32)
            nc.vector.tensor_scalar_mul(rH[:], ps_r[:], ALPHA)
            rF = const.tile([128, layer_reg_len], f32)
            nc.vector.tensor_scalar(rF[:], ps_r[:], -1.0, 1.0, mul, add)

            h16 = const.tile([128, TILES * D], f16)
            nc.vector.tensor_copy(h16[:], feat[:])

            hr = const.tile([128, TILES * D], f16)  # alpha * r_i * h, per hop
            ds = const.tile([128, TILES], f32)

            # AllGather staging: one DRAM buffer + piece tables per hop.
            # piece layout: row = part*HALF_TILES + (tile - 25*half), i.e.
            # partition-major; ag viewed as [128, HALF_TILES, D].
            ag = [
                [
                    dram.tile([128, HALF_TILES * D], f16, name=f"ag{j}_{p}")
                    for p in range(2)
                ]
                for j in range(n_hops)
            ]
            tables = [
                [
                    dram.tile(
                        [PIECE_ROWS, D], f16, addr_space="Shared",
                        name=f"table{j}_{p}",
                    )
                    for p in range(2)
                ]
                for j in range(n_hops)
            ]

            def stage_chunk_dma(j, c, stage_tile):
                # chunk c covers tiles [c*TPC, (c+1)*TPC) in half `piece`
                piece = 0 if c < NCHUNK // 2 else 1
                cc = c - piece * (NCHUNK // 2)
                dst3 = (
                    ag[j][piece][:]
                    .rearrange("p (t e) -> p t e", e=D)[:, cc * TPC : (cc + 1) * TPC]
                )
                src3 = stage_tile[:].rearrange("p (t e) -> p t e", e=D)
                nc.sync.dma_start(dst3, src3)

            def allgather_piece(j, piece):
                nc.gpsimd.collective_compute(
                    "AllGather",
                    mybir.AluOpType.bypass,
                    replica_groups=[list(range(NCORES))],
                    ins=[ag[j][piece].opt()],
                    outs=[tables[j][piece].opt()],
                )

            # hop "-1": initial table = d * h (staged per chunk on Scalar eng)
            if n_hops > 0:
                for c in range(NCHUNK):
                    stage_t = stg.tile([128, TPC * D], f16, tag="stage")
                    for k in range(TPC):
                        t = c * TPC + k
                        nc.scalar.mul(
                            stage_t[:, k * D : (k + 1) * D],
                            feat[:, t * D : (t + 1) * D],
                            dcols[:, t : t + 1],
                        )
                    stage_chunk_dma(0, c, stage_t)
                    if c == NCHUNK // 2 - 1:
                        allgather_piece(0, 0)
                allgather_piece(0, 1)

            for i in range(n_hops):
                nc.vector.tensor_scalar_mul(ds[:], dcols[:], rA[:, i : i + 1])
                nc.scalar.mul(hr[:], h16[:], rH[:, i : i + 1])
                for c in range(NCHUNK):
                    ebuf = ebufs.tile([128, BPC * 128], f16, tag="ebuf")
                    col0 = c * BPC * 128 // 16
                    if skip_gather:
                        nc.vector.memset(ebuf[:], 0.0)
                    else:
                        for off, n_idx, piece, q in gather_plan:
                            nc.gpsimd.dma_gather(
                                out_ap=ebuf[:, off : off + n_idx].rearrange(
                                    "p (g e) -> p g e", e=D
                                ),
                                in_ap=tables[i][piece][:],
                                idxs_ap=idx_sb[
                                    :, col0 + off // 16 : col0 + (off + n_idx) // 16
                                ],
                                num_idxs=n_idx,
                                num_idxs_reg=n_idx,
                                elem_size=D,
                                single_packet=False,
                                queue_num=q,
                            )
                    if i < n_hops - 1:
                        stage_t = stg.tile(
                            [128, TPC * D], f16, tag="stage", name="stage_t"
                        )
                    else:
                        stage_t = None
                    for k in range(TPC):
                        t = c * TPC + k
                        psum = pspool.tile([128, D], f32, tag="ps")
                        if skip_mm:
                            nc.vector.memset(psum[:], 0.0)
                        for j in range([0, BA + BB][not skip_mm]):
                            if j < BA:
                                b = k * BA + j
                            else:
                                b = TPC * BA + k * BB + (j - BA)
                            wb = c * BPC + b
                            nc.tensor.matmul(
                                psum[:],
                                w_sb[:, wb * 128 : (wb + 1) * 128],
                                ebuf[:, b * 128 : (b + 1) * 128],
                                start=(j == 0),
                                stop=(j == BA + BB - 1),
                            )
                        tc0 = t * D
                        u2 = utmps.tile([128, D], f32, tag="u2")
                        nc.vector.scalar_tensor_tensor(
                            u2[:],
                            psum[:],
                            ds[:, t : t + 1],
                            hr[:, tc0 : tc0 + D],
                            mul,
                            add,
                        )
                        nc.vector.scalar_tensor_tensor(
                            feat[:, tc0 : tc0 + D],
                            feat[:, tc0 : tc0 + D],
                            rF[:, i : i + 1],
                            u2[:],
                            mul,
                            add,
                        )
                        if stage_t is not None:
                            nc.scalar.mul(
                                stage_t[:, k * D : (k + 1) * D],
                                feat[:, tc0 : tc0 + D],
                                dcols[:, t : t + 1],
                            )
                    if stage_t is not None:
                        stage_chunk_dma(i + 1, c, stage_t)
                        if c == NCHUNK // 2 - 1:
                            allgather_piece(i + 1, 0)
                        elif c == NCHUNK - 1:
                            allgather_piece(i + 1, 1)

            nc.sync.dma_start(out_feat[:], feat[:])
    nc.finalize()
    return nc


# ---------------------------------------------------------------------------
# Entry point
# ---------------------------------------------------------------------------
def kernel(h, d, layer_reg, src, dst):
    _install_shims()
    from concourse.bass_utils import run_bass_kernel_spmd

    h = np.asarray(h, np.float32)
    d = np.asarray(d, np.float32)
    layer_reg = np.asarray(layer_reg, np.float32)
    src = np.asarray(src, np.int64)
    dst = np.asarray(dst, np.int64)
    n_nodes = h.shape[0]
    shard = n_nodes // NCORES

    per_core, meta = _preprocess(src, dst, n_nodes)
    tile_of, part_of, dev_of = meta["tile_of"], meta["part_of"], meta["dev_of"]

    in_maps = []
    for dev in range(NCORES):
        nodes = np.arange(dev * shard, (dev + 1) * shard)
        tl, pt = tile_of[nodes], part_of[nodes]
        h_shard = np.zeros((128, TILES, D), np.float32)
        h_shard[pt, tl] = h[nodes]
        dcols = np.zeros((128, TILES), np.float32)
        dcols[pt, tl] = d[nodes]
        in_maps.append(
            {
                "idx": per_core[dev]["idx"],
                "w": per_core[dev]["w"],
                "h": h_shard.reshape(128, TILES * D),
                "dcols": dcols,
                "lr": layer_reg.reshape(1, -1),
            }
        )

    import os
    n_hops = int(os.environ.get("APPNP_HOPS", HOPS))
    nc = _build(meta, n_hops, len(layer_reg))
    res = run_bass_kernel_spmd(
        nc, in_maps, list(range(NCORES)), trace=bool(PROFILE.get("trace"))
    )
    PROFILE["exec_time_ns"] = res.exec_time_ns
    PROFILE["results"] = res

    out = np.empty((n_nodes, D), np.float32)
    for dev in range(NCORES):
        nodes = np.arange(dev * shard, (dev + 1) * shard)
        of = res.results[dev]["out_feat"].reshape(128, TILES, D)
        out[nodes] = of[part_of[nodes], tile_of[nodes]]
    return out


# revision 8
# speedup vs baseline: 1.0053x; 1.0053x over previous
"""APPNP propagation (10 hops) on Trainium2, 8 NeuronCores.

Strategy (dst-sharded message passing, deep-pipelined):
- Nodes are sharded over 8 cores by id (6250 dst nodes each). Each core owns
  the incoming edges of its nodes and computes their feature updates.
- Each shard's nodes are split by in-shard id into two halves (3125 nodes),
  packed into tiles 0-24 (half 0) and 25-49 (half 1). The replicated bf16
  "scaled feature" table t[n] = d[n] * feat[n] is split into two pieces
  (one per half, 8*3200 = 25600 rows each, so rows fit int16), AllGathered
  separately: piece 0 fires mid-hop (hidden behind the tail chunks), piece 1
  is the half-size hop tail and overlaps the next hop's half-0 gathers.
- Per hop each core gathers t[src] rows for its edges with 4 dma_gather
  calls per chunk (half-0 edges split across queues 0/1, half-1 across 2/3)
  so all 4 SWDGE queues stay busy, multiplies by per-edge one-hot fp8 weight
  blocks on the PE (segment-sum into PSUM), applies the APPNP update with two
  fused scalar_tensor_tensor ops on the DVE, and stages d*feat (bf16) via the
  otherwise-idle Scalar engine, DMAing each chunk's rows to the AllGather
  input buffer as soon as they are ready.
- Per-core dst tiles are packed so every tile has exactly BA blocks of "A"
  edges (src in half 0) and BB blocks of "B" edges; gather indices are int16
  rows into the corresponding table piece.
"""

import contextlib
import sys
import types

sys.path.insert(0, "/opt/trn_rl_repo")

import numpy as np
import ml_dtypes


# ---------------------------------------------------------------------------
# Environment shims (walrus in this container allows only 1 sync wait per CTRL
# instruction; the image's antenv stub lacks the NTFF profile hook).
# ---------------------------------------------------------------------------
def _install_shims():
    import concourse.mybir as mybir
    import concourse.tile as tile_mod
    from concourse.vector_clock import ScopedClock

    if getattr(tile_mod.TileContext, "_appnp_patched", False):
        return

    def _drain_and_barrier(self, tick_clock, wait_clock):
        nc = self.nc
        probe = nc.sync.nop(nofuse=True)
        wait_clock.add_sem_waits(
            probe.ins, ScopedClock({None: tick_clock.global_clock})
        )
        waits = list(probe.ins.sync_info.on_wait) if probe.ins.sync_info else []
        if probe.ins.sync_info:
            probe.ins.sync_info.on_wait = waits[:1]
        for i in range(1, len(waits)):
            extra = nc.sync.nop(nofuse=True)
            if extra.ins.sync_info is None:
                extra.ins.sync_info = mybir.SyncInfo(
                    on_wait=waits[i : i + 1], on_update=[]
                )
            else:
                extra.ins.sync_info.on_wait = waits[i : i + 1]
        nc.sync.drain()
        nc.all_engine_barrier()
        assert self.sems is not None
        popped = nc._tile_sem_poison_stack.pop()
        assert popped is self._sem_poison
        nc.clear_and_free_semaphores(list(self.sems.allocated().values()))
        nc.all_engine_barrier()

    tile_mod.TileContext._drain_and_barrier = _drain_and_barrier
    tile_mod.TileContext._appnp_patched = True

    import antenv

    if "antenv.axon_hooks" not in sys.modules:
        hooks_mod = types.ModuleType("antenv.axon_hooks")
        _HOOK = [None]
        hooks_mod.set_axon_ntff_profile_hook = lambda h: _HOOK.__setitem__(0, h)
        hooks_mod.get_axon_ntff_profile_hook = lambda: _HOOK[0]
        sys.modules["antenv.axon_hooks"] = hooks_mod
        antenv.axon_hooks = hooks_mod
        try:
            from trn_agent_boot.trn_boot import _ntff_profile_via_ctypes

            hooks_mod.set_axon_ntff_profile_hook(
                _ntff_profile_via_ctypes("/opt/axon/libaxon_pjrt.so")
            )
        except Exception:
            pass

    import concourse.bass_utils as bass_utils

    bass_utils.upload_artifacts = lambda tmpdir: f"file://{tmpdir}"


# ---------------------------------------------------------------------------
# Constants
# ---------------------------------------------------------------------------
NCORES = 8
HOPS = 10
ALPHA = 0.1
D = 128
TILES = 50  # dst tiles per core
HALF_TILES = TILES // 2  # tiles per half
TILES_PER_CHUNK = 5
NCHUNK = TILES // TILES_PER_CHUNK
PIECE_ROWS = NCORES * HALF_TILES * 128  # 25600 rows per table piece

# set by bench harness: {"trace": True} -> records exec_time_ns
PROFILE = {}


# ---------------------------------------------------------------------------
# Host-side graph preprocessing (pure index manipulation)
# ---------------------------------------------------------------------------
def _pack_bins(degA, degB, capA, capB, n_bins, cap_nodes=128):
    """Assign nodes to bins, balancing A and B edge counts. Returns
    (tile_of, part_of) or None if infeasible with the given caps."""
    n = len(degA)
    order = np.argsort(-(degA + degB), kind="stable")
    binsA = np.zeros(n_bins, np.int64)
    binsB = np.zeros(n_bins, np.int64)
    binsN = np.zeros(n_bins, np.int64)
    tile_of = np.zeros(n, np.int32)
    part_of = np.zeros(n, np.int32)
    tA = max(1.0, degA.sum() / n_bins)
    tB = max(1.0, degB.sum() / n_bins)
    for node in order:
        a, b = degA[node], degB[node]
        feas = (binsN < cap_nodes) & (binsA + a <= capA) & (binsB + b <= capB)
        if not feas.any():
            return None
        score = np.maximum((binsA + a) / tA, (binsB + b) / tB)
        score[~feas] = np.inf
        t = int(np.argmin(score))
        tile_of[node] = t
        part_of[node] = binsN[t]
        binsA[t] += a
        binsB[t] += b
        binsN[t] += 1
    return tile_of, part_of


def _preprocess(src, dst, n_nodes):
    shard = n_nodes // NCORES  # 6250
    halfsz = shard // 2  # 3125

    node_ids = np.arange(n_nodes)
    dev_of = (node_ids // shard).astype(np.int32)
    inshard = (node_ids % shard).astype(np.int64)
    half_of = (inshard >= halfsz).astype(np.int32)

    e_dev = dev_of[dst]
    e_isB = half_of[src].astype(bool)  # which table piece the src row lives in

    tile_of = np.zeros(n_nodes, np.int32)  # 0..49 (half 1 -> +25)
    part_of = np.zeros(n_nodes, np.int32)
    packs = []  # (dev, half) -> (degA, degB) over that half's 3125 nodes
    for dev in range(NCORES):
        for h in (0, 1):
            m = (e_dev == dev) & (half_of[dst] == h)
            dl = inshard[dst[m]] - h * halfsz  # 0..halfsz-1
            isB = e_isB[m]
            degA = np.bincount(dl[~isB], minlength=halfsz)
            degB = np.bincount(dl[isB], minlength=halfsz)
            packs.append((degA, degB))

    maxA = max(int(np.ceil(p[0].sum() / HALF_TILES)) for p in packs)
    maxB = max(int(np.ceil(p[1].sum() / HALF_TILES)) for p in packs)
    BA = max(1, (maxA + 127) // 128)
    BB = max(1, (maxB + 127) // 128)
    while True:
        ok = True
        for dev in range(NCORES):
            for h in (0, 1):
                degA, degB = packs[dev * 2 + h]
                r = _pack_bins(degA, degB, BA * 128, BB * 128, HALF_TILES)
                if r is None:
                    ok = False
                    break
                base = dev * shard + h * halfsz
                tile_of[base : base + halfsz] = r[0] + h * HALF_TILES
                part_of[base : base + halfsz] = r[1]
            if not ok:
                break
        if ok:
            break
        # couldn't fit: grow the tighter side
        if BA <= BB:
            BA += 1
        else:
            BB += 1

    # table-piece row (partition-major within a shard's half):
    # row = dev*3200 + part*HALF_TILES + (tile mod HALF_TILES)
    row_of = (
        dev_of.astype(np.int64) * (HALF_TILES * 128)
        + part_of.astype(np.int64) * HALF_TILES
        + (tile_of % HALF_TILES)
    )

    BPC = TILES_PER_CHUNK * (BA + BB)
    nblk = NCHUNK * BPC
    tot_slots = nblk * 128
    per_core = []
    e_srow = row_of[src]
    e_tile = tile_of[dst]
    e_part = part_of[dst]
    for dev in range(NCORES):
        m = e_dev == dev
        tiles_ = e_tile[m]
        isB_ = e_isB[m]
        parts_ = e_part[m]
        srows_ = e_srow[m]
        idx_flat = np.zeros(tot_slots, np.int16)
        w_inblock = []
        w_block = []
        w_dstp = []
        for c in range(NCHUNK):
            for half in (0, 1):  # A (src half 0) then B (src half 1) blocks
                nb = BA if half == 0 else BB
                for k in range(TILES_PER_CHUNK):
                    t = c * TILES_PER_CHUNK + k
                    sel = (tiles_ == t) & (isB_ == bool(half))
                    sr = srows_[sel]
                    pp = parts_[sel]
                    cap = nb * 128
                    assert len(sr) <= cap, (dev, t, half, len(sr), cap)
                    if half == 0:
                        b0 = c * BPC + k * BA
                    else:
                        b0 = c * BPC + TILES_PER_CHUNK * BA + k * BB
                    off = b0 * 128
                    idx_flat[off : off + len(sr)] = sr.astype(np.int16)
                    j = np.arange(len(sr))
                    w_inblock.append((j % 128).astype(np.int64))
                    w_block.append(b0 + j // 128)
                    w_dstp.append(pp.astype(np.int64))
        # wrap indices: slot s -> (partition s%16 [replicated x8], col s//16)
        idx_wrap = np.zeros((128, tot_slots // 16), np.int16)
        for p in range(128):
            idx_wrap[p, :] = idx_flat[p % 16 :: 16]
        w = np.zeros((128, nblk * 128), dtype=ml_dtypes.float8_e4m3)
        w_inblock = np.concatenate(w_inblock)
        w_block = np.concatenate(w_block)
        w_dstp = np.concatenate(w_dstp)
        w[w_inblock, w_block * 128 + w_dstp] = 1.0
        per_core.append({"idx": idx_wrap, "w": w})

    meta = dict(
        BA=BA,
        BB=BB,
        BPC=BPC,
        nblk=nblk,
        tot_slots=tot_slots,
        shard=shard,
        tile_of=tile_of,
        part_of=part_of,
        dev_of=dev_of,
    )
    return per_core, meta


# ---------------------------------------------------------------------------
# Bass kernel build
# ---------------------------------------------------------------------------
def _build(meta, n_hops, layer_reg_len):
    import os
    skip_gather = bool(int(os.environ.get("APPNP_SKIP_GATHER", "0")))
    skip_mm = bool(int(os.environ.get("APPNP_SKIP_MM", "0")))
    import concourse.bacc as bacc
    import concourse.mybir as mybir
    import concourse.tile as tile

    f32, f16, fp8, i16 = (
        mybir.dt.float32,
        mybir.dt.bfloat16,
        mybir.dt.float8e4,
        mybir.dt.int16,
    )
    BA, BB, BPC = meta["BA"], meta["BB"], meta["BPC"]
    nblk, tot_slots = meta["nblk"], meta["tot_slots"]
    TPC = TILES_PER_CHUNK
    mul, add = mybir.AluOpType.mult, mybir.AluOpType.add

    nc = bacc.Bacc(
        "TRN2",
        target_bir_lowering=False,
        debug=False,
        num_devices=NCORES,
        num_swdge_queues=4,
    )
    idx_in = nc.declare_dram_parameter("idx", [128, tot_slots // 16], i16, isOutput=False)
    w_in = nc.declare_dram_parameter("w", [128, nblk * 128], fp8, isOutput=False)
    h_in = nc.declare_dram_parameter("h", [128, TILES * D], f32, isOutput=False)
    dcols_in = nc.declare_dram_parameter("dcols", [128, TILES], f32, isOutput=False)
    lr_in = nc.declare_dram_parameter("lr", [1, layer_reg_len], f32, isOutput=False)
    out_feat = nc.declare_dram_parameter("out_feat", [128, TILES * D], f32, isOutput=True)

    # A-gather split across queues 0/1, B across 2/3 (block-aligned halves)
    nA = TPC * BA * 128
    nB = TPC * BB * 128
    a_splits = [((TPC * BA + 1) // 2) * 128]
    b_splits = [((TPC * BB + 1) // 2) * 128]
    gather_plan = []  # (slot_off, n_idxs, piece, queue)
    gather_plan.append((0, a_splits[0], 0, 0))
    gather_plan.append((a_splits[0], nA - a_splits[0], 0, 1))
    gather_plan.append((nA, b_splits[0], 1, 2))
    gather_plan.append((nA + b_splits[0], nB - b_splits[0], 1, 3))

    with tile.TileContext(nc) as tc:
        with (
            tc.tile_pool(name="const", bufs=1) as const,
            tc.tile_pool(name="ebufs", bufs=3) as ebufs,
            tc.tile_pool(name="stg", bufs=2) as stg,
            tc.tile_pool(name="hrp", bufs=3) as hrp,
            tc.tile_pool(name="dsp", bufs=2) as dsp,
            tc.tile_pool(name="utmps", bufs=4) as utmps,
            tc.tile_pool(name="ps", bufs=6, space="PSUM") as pspool,
            tc.tile_pool(name="dram", bufs=1, space="DRAM") as dram,
        ):
            idx_sb = const.tile([128, tot_slots // 16], i16)
            nc.sync.dma_start(idx_sb[:], idx_in[:])
            w_sb = const.tile([128, nblk * 128], fp8)
            nc.sync.dma_start(w_sb[:], w_in[:])
            feat = const.tile([128, TILES * D], f32)
            nc.sync.dma_start(feat[:], h_in[:])
            dcols = const.tile([128, TILES], f32)
            nc.sync.dma_start(dcols[:], dcols_in[:])
            lr_sb = const.tile([1, layer_reg_len], f32)
            nc.sync.dma_start(lr_sb[:], lr_in[:])

            # broadcast layer_reg across partitions via K=1 matmul with ones
            ones = const.tile([1, 128], f32)
            nc.vector.memset(ones[:], 1.0)
            ps_r = pspool.tile([128, layer_reg_len], f32, tag="psr", bufs=1)
            nc.tensor.matmul(ps_r[:], ones[:], lr_sb[:], start=True, stop=True)
            rA = const.tile([128, layer_reg_len], f32)
            nc.vector.tensor_scalar_mul(rA[:], ps_r[:], 1.0 - ALPHA)
            rH = const.tile([128, layer_reg_len], f32)
            nc.vector.tensor_scalar_mul(rH[:], ps_r[:], ALPHA)
            rF = const.tile([128, layer_reg_len], f32)
            nc.vector.tensor_scalar(rF[:], ps_r[:], -1.0, 1.0, mul, add)

            h16 = const.tile([128, TILES * D], f16)
            nc.vector.tensor_copy(h16[:], feat[:])

            # AllGather staging: one DRAM buffer + piece tables per hop.
            # piece layout: row = part*HALF_TILES + (tile - 25*half), i.e.
            # partition-major; ag viewed as [128, HALF_TILES, D].
            ag = [
                [
                    dram.tile([128, HALF_TILES * D], f16, name=f"ag{j}_{p}")
                    for p in range(2)
                ]
                for j in range(n_hops)
            ]
            tables = [
                [
                    dram.tile(
                        [PIECE_ROWS, D], f16, addr_space="Shared",
                        name=f"table{j}_{p}",
                    )
                    for p in range(2)
                ]
                for j in range(n_hops)
            ]

            def stage_chunk_dma(j, c, stage_tile):
                # chunk c covers tiles [c*TPC, (c+1)*TPC) in half `piece`
                piece = 0 if c < NCHUNK // 2 else 1
                cc = c - piece * (NCHUNK // 2)
                dst3 = (
                    ag[j][piece][:]
                    .rearrange("p (t e) -> p t e", e=D)[:, cc * TPC : (cc + 1) * TPC]
                )
                src3 = stage_tile[:].rearrange("p (t e) -> p t e", e=D)
                nc.sync.dma_start(dst3, src3)

            def allgather_piece(j, piece):
                nc.gpsimd.collective_compute(
                    "AllGather",
                    mybir.AluOpType.bypass,
                    replica_groups=[list(range(NCORES))],
                    ins=[ag[j][piece].opt()],
                    outs=[tables[j][piece].opt()],
                )

            # hop "-1": initial table = d * h (staged per chunk on Scalar eng)
            if n_hops > 0:
                for c in range(NCHUNK):
                    stage_t = stg.tile([128, TPC * D], f16, tag="stage")
                    for k in range(TPC):
                        t = c * TPC + k
                        nc.scalar.mul(
                            stage_t[:, k * D : (k + 1) * D],
                            feat[:, t * D : (t + 1) * D],
                            dcols[:, t : t + 1],
                        )
                    stage_chunk_dma(0, c, stage_t)
                    if c == NCHUNK // 2 - 1:
                        allgather_piece(0, 0)
                allgather_piece(0, 1)

            for i in range(n_hops):
                ds = dsp.tile([128, TILES], f32, tag="ds", name="ds")
                nc.vector.tensor_scalar_mul(ds[:], dcols[:], rA[:, i : i + 1])
                for c in range(NCHUNK):
                    ebuf = ebufs.tile([128, BPC * 128], f16, tag="ebuf")
                    col0 = c * BPC * 128 // 16
                    if skip_gather:
                        nc.vector.memset(ebuf[:], 0.0)
                    else:
                        for off, n_idx, piece, q in gather_plan:
                            nc.gpsimd.dma_gather(
                                out_ap=ebuf[:, off : off + n_idx].rearrange(
                                    "p (g e) -> p g e", e=D
                                ),
                                in_ap=tables[i][piece][:],
                                idxs_ap=idx_sb[
                                    :, col0 + off // 16 : col0 + (off + n_idx) // 16
                                ],
                                num_idxs=n_idx,
                                num_idxs_reg=n_idx,
                                elem_size=D,
                                single_packet=False,
                                queue_num=q,
                            )
                    if i < n_hops - 1:
                        stage_t = stg.tile(
                            [128, TPC * D], f16, tag="stage", name="stage_t"
                        )
                    else:
                        stage_t = None
                    # alpha * r_i * h for this chunk (Scalar engine)
                    hr_t = hrp.tile([128, TPC * D], f16, tag="hr", name="hr_t")
                    nc.scalar.mul(
                        hr_t[:],
                        h16[:, c * TPC * D : (c + 1) * TPC * D],
                        rH[:, i : i + 1],
                    )
                    for k in range(TPC):
                        t = c * TPC + k
                        psum = pspool.tile([128, D], f32, tag="ps")
                        if skip_mm:
                            nc.vector.memset(psum[:], 0.0)
                        for j in range([0, BA + BB][not skip_mm]):
                            if j < BA:
                                b = k * BA + j
                            else:
                                b = TPC * BA + k * BB + (j - BA)
                            wb = c * BPC + b
                            nc.tensor.matmul(
                                psum[:],
                                w_sb[:, wb * 128 : (wb + 1) * 128],
                                ebuf[:, b * 128 : (b + 1) * 128],
                                start=(j == 0),
                                stop=(j == BA + BB - 1),
                            )
                        tc0 = t * D
                        u2 = utmps.tile([128, D], f32, tag="u2")
                        nc.vector.scalar_tensor_tensor(
                            u2[:],
                            psum[:],
                            ds[:, t : t + 1],
                            hr_t[:, k * D : (k + 1) * D],
                            mul,
                            add,
                        )
                        nc.vector.scalar_tensor_tensor(
                            feat[:, tc0 : tc0 + D],
                            feat[:, tc0 : tc0 + D],
                            rF[:, i : i + 1],
                            u2[:],
                            mul,
                            add,
                        )
                        if stage_t is not None:
                            nc.scalar.mul(
                                stage_t[:, k * D : (k + 1) * D],
                                feat[:, tc0 : tc0 + D],
                                dcols[:, t : t + 1],
                            )
                    if stage_t is not None:
                        stage_chunk_dma(i + 1, c, stage_t)
                if i < n_hops - 1:
                    allgather_piece(i + 1, 0)
                    allgather_piece(i + 1, 1)

            nc.sync.dma_start(out_feat[:], feat[:])
    nc.finalize()
    return nc


# ---------------------------------------------------------------------------
# Entry point
# ---------------------------------------------------------------------------
def kernel(h, d, layer_reg, src, dst):
    _install_shims()
    from concourse.bass_utils import run_bass_kernel_spmd

    h = np.asarray(h, np.float32)
    d = np.asarray(d, np.float32)
    layer_reg = np.asarray(layer_reg, np.float32)
    src = np.asarray(src, np.int64)
    dst = np.asarray(dst, np.int64)
    n_nodes = h.shape[0]
    shard = n_nodes // NCORES

    per_core, meta = _preprocess(src, dst, n_nodes)
    tile_of, part_of, dev_of = meta["tile_of"], meta["part_of"], meta["dev_of"]

    in_maps = []
    for dev in range(NCORES):
        nodes = np.arange(dev * shard, (dev + 1) * shard)
        tl, pt = tile_of[nodes], part_of[nodes]
        h_shard = np.zeros((128, TILES, D), np.float32)
        h_shard[pt, tl] = h[nodes]
        dcols = np.zeros((128, TILES), np.float32)
        dcols[pt, tl] = d[nodes]
        in_maps.append(
            {
                "idx": per_core[dev]["idx"],
                "w": per_core[dev]["w"],
                "h": h_shard.reshape(128, TILES * D),
                "dcols": dcols,
                "lr": layer_reg.reshape(1, -1),
            }
        )

    import os
    n_hops = int(os.environ.get("APPNP_HOPS", HOPS))
    nc = _build(meta, n_hops, len(layer_reg))
    res = run_bass_kernel_spmd(
        nc, in_maps, list(range(NCORES)), trace=bool(PROFILE.get("trace"))
    )
    PROFILE["exec_time_ns"] = res.exec_time_ns
    PROFILE["results"] = res

    out = np.empty((n_nodes, D), np.float32)
    for dev in range(NCORES):
        nodes = np.arange(dev * shard, (dev + 1) * shard)
        of = res.results[dev]["out_feat"].reshape(128, TILES, D)
        out[nodes] = of[part_of[nodes], tile_of[nodes]]
    return out
